# revision 18
# baseline (speedup 1.0000x reference)
"""BiGraphSAGEDecoder Trainium2 kernel (bf16 rewrite).

Sharding: 8 cores = 4 batches x {up-path, down-path}. One SPMD bass program;
the up/down asymmetry is handled purely by data (down cores receive host-
transposed adjacency / adjacency-weight matrices). Per layer, the two cores of
a batch exchange their halves of the concatenated features with a 2-rank
AllGather, then each normalizes redundantly so both hold the full h.

Key implementation choices vs the fp32 baseline:
  - all matrix operands in bf16 (PE rate is the same 1 cycle/row, but DMA
    bytes halve and the DVE/scalar elementwise work halves)
  - adj / adj-weight stream in as whole [128, N] row blocks (4KB contiguous
    rows) and the masked product prod = adj .* Wadj is held fully SBUF
    resident per layer
  - mm2 output xT stays in SBUF (no DRAM roundtrip before mm3)
  - layer 2 never forms s = prod^T @ h: only rows {N-2,N-1} of x are needed,
    so compute vT = u^T @ prod (u = invT[:, N-2:N]), then x2 = v^T @ h
  - normalize+leaky fused on the scalar engine (Square+accum, Lrelu w/ scale)
  - emission order software-pipelined (A=stream, B=compute, C=assemble):
    A0 B0 A1 C0 B1 A2 C1 B2 so next-layer streaming never queues behind
    AllGather-dependent instructions on any engine queue
"""

import os
import sys
import types
import contextlib

sys.path.insert(0, "/opt/trn_rl_repo")

import numpy as np
import ml_dtypes

import concourse.bass as bass
import concourse.tile as tile
from concourse import mybir, bacc
from concourse.mybir import AxisListType
from concourse.masks import make_identity
from concourse.bass_utils import run_bass_kernel_spmd

FP = mybir.dt.float32
BF = mybir.dt.float16
AF = mybir.ActivationFunctionType
ALU = mybir.AluOpType

# ---------------------------------------------------------------------------
# Environment patches (required for this container's toolchain)
# ---------------------------------------------------------------------------


def install_ntff_shim():
    """antenv.axon_hooks is absent in this image; provide it so trace=True
    profiling works (used by test.py, harmless otherwise)."""
    try:
        import antenv.axon_hooks  # noqa: F401
        return
    except ImportError:
        pass
    try:
        import antenv
    except ImportError:
        return
    mod = types.ModuleType("antenv.axon_hooks")
    _holder = {"hook": None}
    mod.set_axon_ntff_profile_hook = lambda h: _holder.__setitem__("hook", h)
    mod.get_axon_ntff_profile_hook = lambda: _holder["hook"]
    sys.modules["antenv.axon_hooks"] = mod
    antenv.axon_hooks = mod
    try:
        from trn_agent_boot.trn_boot import _ntff_profile_via_ctypes

        hook = _ntff_profile_via_ctypes("/opt/axon/libaxon_pjrt.so")
        if hook is not None:
            mod.set_axon_ntff_profile_hook(hook)
    except Exception:
        pass


install_ntff_shim()

if os.environ.get("KGSD_LDW_OPT", "0") != "0":
    # NOTE: walrus rejects ldweights-dedup'd fp16 matmuls (visitInstLdweights
    # error), so this stays off by default for the fp16 kernel.
    # let walrus dedup back-to-back LDWEIGHTS
    import concourse.bass_utils as _bu
    _orig_run_command = _bu.run_command

    def _patched_run_command(argv, **kw):
        argv = ["--enable-ldw-opt=true" if a == "--enable-ldw-opt=false"
                else a for a in argv]
        return _orig_run_command(argv, **kw)

    _bu.run_command = _patched_run_command

# ---------------------------------------------------------------------------
# Problem constants
# ---------------------------------------------------------------------------

N_FULL = 2048
B = 4
P = 128
DOUT = 256     # per-path cat chunk width
BH = 128       # bias half width per core
DEC = 128
D3 = 3 * DOUT  # full cat width (768)
DINS = (256, 768, 768)   # per-layer input dims
EPS = 1e-12
LEAK = 0.1
MM2_JP = 512   # mm2 output column superblock


# ---------------------------------------------------------------------------
# Program builder
# ---------------------------------------------------------------------------

def build_program(n_cores: int, N: int = N_FULL, stop_phase: int = 99,
                  dbg: bool = False):
    """Build the SPMD bass program. Returns nc."""
    NT = N // P            # 128-row tiles
    NCH = 4 if NT % 4 == 0 else 2   # exchange chunks
    NCT = NT // NCH        # tiles per exchange chunk
    NJP = N // MM2_JP      # mm2 column superblocks
    NV = N // 512          # vT column superblocks (layer 2)

    nc = bacc.Bacc("TRN2", target_bir_lowering=False, debug=False,
                   num_devices=n_cores)

    # --- DRAM I/O ---
    x_d = nc.dram_tensor("x", [N, DINS[0]], BF, kind="ExternalInput")
    adj_d = nc.dram_tensor("adj", [N, N], BF, kind="ExternalInput")
    adj2_d = nc.dram_tensor("adj2", [N, N], BF, kind="ExternalInput")
    invT_d = nc.dram_tensor("invT", [N, N], BF, kind="ExternalInput")
    wa_d = [nc.dram_tensor(f"w{l}a", [N, N], BF, kind="ExternalInput")
            for l in range(3)]
    wc_d = [nc.dram_tensor(f"w{l}c", [DINS[l], DOUT], BF, kind="ExternalInput")
            for l in range(3)]
    wb_d = [nc.dram_tensor(f"w{l}b", [DINS[l], BH], BF, kind="ExternalInput")
            for l in range(3)]
    p1_d = nc.dram_tensor("p1", [D3, DEC], FP, kind="ExternalInput")
    p2_d = nc.dram_tensor("p2", [DEC, DEC], FP, kind="ExternalInput")
    y_d = nc.dram_tensor("ypred", [1, 1], FP, kind="ExternalOutput")
    if dbg:
        dbg_s0 = nc.dram_tensor("dbg_s0", [P, (N // P) * DINS[0]], BF,
                                kind="ExternalOutput")
        dbg_h = [nc.dram_tensor(f"dbg_h{l}", [N, D3], BF,
                                kind="ExternalOutput") for l in range(2)]
        dbg_hd = nc.dram_tensor("dbg_hd", [2, D3], BF, kind="ExternalOutput")
        dbg_vT = nc.dram_tensor("dbg_vT", [2, N], BF, kind="ExternalOutput")
        dbg_x2 = nc.dram_tensor("dbg_x2", [2, DINS[2]], BF,
                                kind="ExternalOutput")
        dbg_dr = nc.dram_tensor("dbg_dr", [2, D3], FP, kind="ExternalOutput")

    groups = [[i, i + 1] for i in range(0, n_cores, 2)]
    ND3 = D3 // P

    with tile.TileContext(nc) as tc:
      with contextlib.ExitStack() as ctx:
        const_p = ctx.enter_context(tc.tile_pool(name="const", bufs=1))
        h_p = ctx.enter_context(tc.tile_pool(name="h", bufs=1))
        prod_p = ctx.enter_context(tc.tile_pool(name="prod", bufs=1))
        adjw_p = ctx.enter_context(tc.tile_pool(name="adjw", bufs=2))
        shT_p = ctx.enter_context(tc.tile_pool(name="shT", bufs=1))
        xT_p = ctx.enter_context(tc.tile_pool(name="xT", bufs=2))
        wcb_p = ctx.enter_context(tc.tile_pool(name="wcb", bufs=2))
        inv_p = ctx.enter_context(tc.tile_pool(name="invs", bufs=6))
        misc_p = ctx.enter_context(tc.tile_pool(name="misc", bufs=2))
        norm_p = ctx.enter_context(tc.tile_pool(name="norm", bufs=2))
        psum_p = ctx.enter_context(
            tc.tile_pool(name="psum", bufs=8, space="PSUM"))
        dram_p = ctx.enter_context(
            tc.tile_pool(name="dram", bufs=2, space="DRAM"))

        ident = const_p.tile([P, P], BF, tag="identbf")
        make_identity(nc, ident)
        identf = const_p.tile([P, P], FP, tag="identf")
        make_identity(nc, identf)

        # head weights (loaded once, used at the very end)
        p1_t = const_p.tile([P, ND3 * DEC], FP, tag="p1")
        for d in range(ND3):
            nc.gpsimd.dma_start(p1_t[:, d * DEC:(d + 1) * DEC],
                                p1_d.ap()[d * P:(d + 1) * P, :])
        p2_t = const_p.tile([P, DEC], FP, tag="p2")
        nc.gpsimd.dma_start(p2_t[:], p2_d.ap())

        def new_h():
            return [h_p.tile([P, D3], BF, tag=f"h{k}", name="h_t")
                    for k in range(NT)]

        h_t = new_h()
        for k in range(NT):
            nc.sync.dma_start(h_t[k][:, 0:DINS[0]],
                              x_d.ap()[k * P:(k + 1) * P, :])
        # u = invT[:, N-2:N] for the layer-2 shortcut; loaded up-front so it
        # never queues behind AllGather triggers
        u_t0 = misc_p.tile([P, NT * 2], BF, tag="ut", bufs=1)
        for k in range(NT):
            nc.gpsimd.dma_start(u_t0[:, k * 2:(k + 1) * 2],
                                invT_d.ap()[k * P:(k + 1) * P, N - 2:N])

        # drain PSUM->SBUF alternating between scalar and vector engines
        def drain(i, dst, src):
            if i % 2 == 0:
                nc.scalar.copy(dst, src)
            else:
                nc.vector.tensor_copy(dst, src)

        # fused L2-normalize + leaky relu over rows of `ap` ([rows, D3])
        def norm_lrelu(ap, rows=P, i=0):
            ssq = norm_p.tile([P, 1], FP, tag="ssq", name="ssq",
                              bufs=4)[0:rows, :]
            sq = norm_p.tile([P, D3], FP, tag="sq", name="sq",
                             bufs=2)[0:rows, :]
            nc.scalar.activation(sq, ap, AF.Square, accum_out=ssq)
            nrm = norm_p.tile([P, 1], FP, tag="nrm", name="nrm",
                              bufs=4)[0:rows, :]
            nc.scalar.activation(nrm, ssq, AF.Sqrt)
            rn = norm_p.tile([P, 1], FP, tag="rn", name="rn",
                             bufs=4)[0:rows, :]
            nc.vector.reciprocal(rn, nrm)
            nc.vector.tensor_scalar(ap, ap, rn, None, ALU.mult)
            lk = norm_p.tile([P, D3], BF, tag="lk", name="lk",
                             bufs=4)[0:rows, :]
            nc.scalar.mul(lk, ap, LEAK)
            nc.vector.tensor_max(ap, ap, lk)

        st8 = {"hdrug": None, "drug_rows": None}

        # =========== phase A: stream weights + adj/wadj, build prod =======
        def phase_a(l):
            din = DINS[l]
            ND = din // P
            wc_t = wcb_p.tile([P, ND * DOUT], BF, tag="wc", name="wc_t")
            for d in range(ND):
                nc.sync.dma_start(
                    wc_t[:, d * DOUT:(d + 1) * DOUT],
                    wc_d[l].ap()[d * P:(d + 1) * P, :])
            wb_t = wcb_p.tile([P, ND * BH], BF, tag="wb", name="wb_t")
            for d in range(ND):
                nc.scalar.dma_start(
                    wb_t[:, d * BH:(d + 1) * BH],
                    wb_d[l].ap()[d * P:(d + 1) * P, :])
            u_t = u_t0 if l == 2 else None
            prods = []
            adj_src = adj2_d if l == 2 else adj_d
            # rotate adj/wa across the three DMA-capable queues so the
            # product stream is not paced by a single queue's bandwidth
            qrot = [(nc.sync, nc.scalar), (nc.gpsimd, nc.sync),
                    (nc.scalar, nc.gpsimd)]
            for k in range(NT):
                qa, qw = qrot[k % 3]
                a_t = adjw_p.tile([P, N], BF, tag="adjst", name="a_t")
                qa.dma_start(a_t[:], adj_src.ap()[k * P:(k + 1) * P, :])
                w_t = adjw_p.tile([P, N], BF, tag="wast", name="w_t")
                qw.dma_start(w_t[:], wa_d[l].ap()[k * P:(k + 1) * P, :])
                pr = prod_p.tile([P, N], BF, tag=f"prod{k}", name="pr")
                nc.vector.tensor_tensor(pr[:], a_t[:], w_t[:], ALU.mult)
                prods.append(pr)
            return {"wc": wc_t, "wb": wb_t, "u": u_t, "prods": prods}

        # =========== phase B: bias, mm1..mm3, stage + AG trigger ==========
        def phase_b(l, a, h_t):
            din = DINS[l]
            ND = din // P
            wc_t, wb_t, prods = a["wc"], a["wb"], a["prods"]

            if l < 2:
                stage_h = [dram_p.tile([NCT * P, DOUT + BH], BF,
                                       tag=f"stage{cc}", name="stage_h")
                           for cc in range(NCH)]

                # ---- bias chunk: hT = h^T (PE), bias = h @ Wb_half ----
                # it-outer so each h tile is consumed (transposed + bias
                # matmul + staged) as soon as its assembly/norm completes
                hT_t = shT_p.tile([P, ND * N], BF, tag="sht", name="hT_t")
                for it in range(NT):
                    for d in range(ND):
                        pt = psum_p.tile([P, P], BF, tag="ps", name="pt")
                        nc.tensor.transpose(
                            pt[:], h_t[it][:, d * P:(d + 1) * P], ident[:])
                        drain(it * ND + d,
                              hT_t[:, d * N + it * P:d * N + (it + 1) * P],
                              pt[:])
                    pb = psum_p.tile([P, BH], FP, tag="ps", name="pb")
                    for d in range(ND):
                        nc.tensor.matmul(
                            pb[:],
                            hT_t[:, d * N + it * P:d * N + (it + 1) * P],
                            wb_t[:, d * BH:(d + 1) * BH],
                            start=(d == 0), stop=(d == ND - 1))
                    sb = misc_p.tile([P, BH], BF, tag="stgb", name="sb",
                                     bufs=4)
                    drain(it, sb[:], pb[:])
                    cc, io = divmod(it, NCT)
                    (nc.scalar, nc.sync, nc.gpsimd)[it % 3].dma_start(
                        stage_h[cc][io * P:(io + 1) * P, DOUT:DOUT + BH],
                        sb[:])

                # ---- mm1: s = prod^T @ h  (s: [N, din], j rows) ----
                s_t = shT_p.tile([P, NT * din], BF, tag="sht", name="s_t")
                wA = min(din, 512)
                if din <= 256:
                    # k-outer in two j-group passes of 8 accumulators (one
                    # PSUM bank each): the PE consumes each prod block as it
                    # arrives instead of slow-walking the DMA stream
                    JG = max(NT // 8, 1)
                    for jg in range(JG):
                        js = [j for j in range(NT) if j % JG == jg]
                        pjs = {j: psum_p.tile([P, din], FP, tag="ps",
                                              name="pj") for j in js}
                        for k in range(NT):
                            for j in js:
                                nc.tensor.matmul(
                                    pjs[j][:], prods[k][:, j * P:(j + 1) * P],
                                    h_t[k][:, 0:din],
                                    start=(k == 0), stop=(k == NT - 1))
                        for j in js:
                            drain(j, s_t[:, j * din:(j + 1) * din], pjs[j][:])
                else:
                  for j in range(NT):
                    pA = psum_p.tile([P, wA], FP, tag="ps", name="pA")
                    pB = psum_p.tile([P, din - 512], FP, tag="ps",
                                     name="pB") if din > 512 else None
                    for k in range(NT):
                        lhsT = prods[k][:, j * P:(j + 1) * P]
                        st = (k == 0)
                        sp = (k == NT - 1)
                        nc.tensor.matmul(pA[:], lhsT, h_t[k][:, 0:wA],
                                         start=st, stop=sp)
                        if pB is not None:
                            nc.tensor.matmul(pB[:], lhsT, h_t[k][:, 512:din],
                                             start=st, stop=sp)
                    drain(j, s_t[:, j * din:j * din + wA], pA[:])
                    if pB is not None:
                        drain(j + 1, s_t[:, j * din + 512:(j + 1) * din],
                              pB[:])

                if dbg and l == 0:
                    nc.sync.dma_start(dbg_s0.ap(), s_t[:])

                # ---- mm2: xT = s^T @ invT (SBUF), mm3: cat = x @ Wc ----
                for jp in range(NJP):
                    pxs = [psum_p.tile([P, MM2_JP], FP, tag="ps", name="px")
                           for _ in range(ND)]
                    for jt in range(NT):
                        r_t = inv_p.tile([P, MM2_JP], BF, tag="inv",
                                         name="r_t")
                        rq = (nc.sync, nc.scalar, nc.gpsimd)[jt % 3]
                        rq.dma_start(
                            r_t[:],
                            invT_d.ap()[jt * P:(jt + 1) * P,
                                        jp * MM2_JP:(jp + 1) * MM2_JP])
                        for d in range(ND):
                            nc.tensor.matmul(
                                pxs[d][:],
                                s_t[:, jt * din + d * P:
                                    jt * din + (d + 1) * P],
                                r_t[:],
                                start=(jt == 0), stop=(jt == NT - 1))
                    xts = []
                    for d in range(ND):
                        xt = xT_p.tile([P, MM2_JP], BF, tag=f"xt{d}",
                                       name="xt")
                        drain(d, xt[:], pxs[d][:])
                        xts.append(xt)
                    for il in range(MM2_JP // P):
                        it = jp * (MM2_JP // P) + il
                        pc = psum_p.tile([P, DOUT], FP, tag="ps", name="pc")
                        for d in range(ND):
                            nc.tensor.matmul(
                                pc[:], xts[d][:, il * P:(il + 1) * P],
                                wc_t[:, d * DOUT:(d + 1) * DOUT],
                                start=(d == 0), stop=(d == ND - 1))
                        sc = misc_p.tile([P, DOUT], BF, tag="stgc", name="sc",
                                         bufs=4)
                        drain(il, sc[:], pc[:])
                        cc, io = divmod(it, NCT)
                        (nc.scalar, nc.sync, nc.gpsimd)[it % 3].dma_start(
                            stage_h[cc][io * P:(io + 1) * P, 0:DOUT], sc[:])

                # ---- exchange (NCH chunks, overlapped with mm2/mm3) ----
                ag_h = []
                for cc in range(NCH):
                    agt = dram_p.tile([2, NCT * P, DOUT + BH], BF,
                                      tag=f"ag{cc}", name="ag_h")
                    nc.gpsimd.collective_compute(
                        "AllGather", ALU.bypass, replica_groups=groups,
                        ins=[stage_h[cc].opt()], outs=[agt.opt()])
                    ag_h.append(agt)
                return {"ag": ag_h}

            # ================= layer 2: only drug rows =================
            hdrug, u_t = st8["hdrug"], a["u"]
            stage2 = dram_p.tile([2, DOUT + BH], FP, tag="stage2")

            # vT = u^T @ prod   ([2, N]) — needs only prods+u, so first
            vT_sb = misc_p.tile([2, N], BF, tag="vT", bufs=1)
            for jv in range(NV):
                pv = psum_p.tile([2, 512], FP, tag="ps", name="pv")
                for k in range(NT):
                    nc.tensor.matmul(
                        pv[:], u_t[:, k * 2:(k + 1) * 2],
                        prods[k][:, jv * 512:(jv + 1) * 512],
                        start=(k == 0), stop=(k == NT - 1))
                nc.vector.tensor_copy(vT_sb[:, jv * 512:(jv + 1) * 512],
                                      pv[:])
            if dbg:
                nc.sync.dma_start(dbg_vT.ap(), vT_sb[:])
            # v ([N, 2] as NT column pairs)
            v_t = misc_p.tile([P, NT * 2], BF, tag="vt", bufs=1)
            for i in range(NT):
                pt = psum_p.tile([P, 2], BF, tag="ps", name="ptv")
                nc.tensor.transpose(pt[:], vT_sb[:, i * P:(i + 1) * P],
                                    ident[0:2, 0:2])
                nc.vector.tensor_copy(v_t[:, i * 2:(i + 1) * 2], pt[:])
            # x2 = v^T @ h   ([2, din]) — consumes h tiles as they assemble
            pxA = psum_p.tile([2, 512], FP, tag="ps", name="pxA")
            pxB = psum_p.tile([2, din - 512], FP, tag="ps", name="pxB")
            for i in range(NT):
                lhsT = v_t[:, i * 2:(i + 1) * 2]
                st = (i == 0)
                sp = (i == NT - 1)
                nc.tensor.matmul(pxA[:], lhsT, h_t[i][:, 0:512],
                                 start=st, stop=sp)
                nc.tensor.matmul(pxB[:], lhsT, h_t[i][:, 512:din],
                                 start=st, stop=sp)
            x2 = misc_p.tile([2, din], BF, tag="x2", bufs=1)
            nc.vector.tensor_copy(x2[:, 0:512], pxA[:])
            nc.vector.tensor_copy(x2[:, 512:din], pxB[:])
            if dbg:
                nc.sync.dma_start(dbg_x2.ap(), x2[:])
            # cat chunk = x2 @ Wc
            x2T = misc_p.tile([P, ND * 2], BF, tag="x2T", bufs=1)
            for d in range(ND):
                pt = psum_p.tile([P, 2], BF, tag="ps", name="ptx")
                nc.tensor.transpose(pt[:], x2[:, d * P:(d + 1) * P],
                                    ident[0:2, 0:2])
                nc.vector.tensor_copy(x2T[:, d * 2:(d + 1) * 2], pt[:])
            pc2 = psum_p.tile([2, DOUT], FP, tag="ps", name="pc2")
            for d in range(ND):
                nc.tensor.matmul(pc2[:], x2T[:, d * 2:(d + 1) * 2],
                                 wc_t[:, d * DOUT:(d + 1) * DOUT],
                                 start=(d == 0), stop=(d == ND - 1))
            sc2 = misc_p.tile([2, DOUT], FP, tag="sc2", bufs=1)
            nc.scalar.copy(sc2[:], pc2[:])
            nc.scalar.dma_start(stage2[:, 0:DOUT], sc2[:])

            # bias chunk from hdrug
            hdT = misc_p.tile([P, ND * 2], BF, tag="hdT", bufs=1)
            for d in range(ND):
                pt = psum_p.tile([P, 2], BF, tag="ps", name="pt2")
                nc.tensor.transpose(pt[:], hdrug[:, d * P:(d + 1) * P],
                                    ident[0:2, 0:2])
                nc.vector.tensor_copy(hdT[:, d * 2:(d + 1) * 2], pt[:])
            pb2 = psum_p.tile([2, BH], FP, tag="ps", name="pb2")
            for d in range(ND):
                nc.tensor.matmul(pb2[:], hdT[:, d * 2:(d + 1) * 2],
                                 wb_t[:, d * BH:(d + 1) * BH],
                                 start=(d == 0), stop=(d == ND - 1))
            sb2 = misc_p.tile([2, BH], FP, tag="sb2", bufs=1)
            nc.scalar.copy(sb2[:], pb2[:])
            nc.scalar.dma_start(stage2[:, DOUT:DOUT + BH], sb2[:])

            ag2 = dram_p.tile([2, 2, DOUT + BH], FP, tag="ag2")
            nc.gpsimd.collective_compute(
                "AllGather", ALU.bypass, replica_groups=groups,
                ins=[stage2.opt()], outs=[ag2.opt()])

            dr = misc_p.tile([2, D3], FP, tag="drug", bufs=1)
            nc.gpsimd.dma_start(dr[:, 0:DOUT], ag2[0, :, 0:DOUT])
            nc.gpsimd.dma_start(dr[:, DOUT:2 * DOUT], ag2[1, :, 0:DOUT])
            nc.gpsimd.dma_start(dr[:, 2 * DOUT:2 * DOUT + BH],
                                ag2[0, :, DOUT:DOUT + BH])
            nc.gpsimd.dma_start(dr[:, 2 * DOUT + BH:D3],
                                ag2[1, :, DOUT:DOUT + BH])
            norm_lrelu(dr[:], rows=2)
            if dbg:
                nc.sync.dma_start(dbg_dr.ap(), dr[:])
            st8["drug_rows"] = dr
            return {}

        # =========== phase C: read AG, assemble + normalize new h =========
        def phase_c(l, b):
            ag_h = b["ag"]
            h_t = new_h()
            for ch in range(NCH):
                asb = []
                for r in range(2):
                    at = misc_p.tile([P, NCT, DOUT + BH], BF,
                                     tag=f"agsb{r}", name="at", bufs=2)
                    nc.gpsimd.dma_start(
                        at[:],
                        ag_h[ch][r].rearrange("(a p) c -> p a c", p=P))
                    asb.append(at)
                for io in range(NCT):
                    it = ch * NCT + io
                    ht = h_t[it]
                    drain(it, ht[:, 0:DOUT], asb[0][:, io, 0:DOUT])
                    drain(it + 1, ht[:, DOUT:2 * DOUT], asb[1][:, io, 0:DOUT])
                    drain(it, ht[:, 2 * DOUT:2 * DOUT + BH],
                          asb[0][:, io, DOUT:DOUT + BH])
                    drain(it + 1, ht[:, 2 * DOUT + BH:D3],
                          asb[1][:, io, DOUT:DOUT + BH])
                    norm_lrelu(ht[:], i=it)
                    if dbg:
                        nc.sync.dma_start(
                            dbg_h[l].ap()[it * P:(it + 1) * P, :], ht[:])

            if l == 1:
                # partition-0-based copy of the two drug rows (PE ops cannot
                # address partitions 126:128); re-normalized separately from
                # the raw (pre-norm) AG output
                hdrug = misc_p.tile([2, D3], BF, tag="hdrug", bufs=1)
                agl = ag_h[NCH - 1]
                NH2 = NCT * P
                nc.gpsimd.dma_start(hdrug[:, 0:DOUT],
                                    agl[0, NH2 - 2:NH2, 0:DOUT])
                nc.gpsimd.dma_start(hdrug[:, DOUT:2 * DOUT],
                                    agl[1, NH2 - 2:NH2, 0:DOUT])
                nc.gpsimd.dma_start(hdrug[:, 2 * DOUT:2 * DOUT + BH],
                                    agl[0, NH2 - 2:NH2, DOUT:DOUT + BH])
                nc.gpsimd.dma_start(hdrug[:, 2 * DOUT + BH:D3],
                                    agl[1, NH2 - 2:NH2, DOUT:DOUT + BH])
                norm_lrelu(hdrug[:], rows=2)
                if dbg:
                    nc.sync.dma_start(dbg_hd.ap(), hdrug[:])
                st8["hdrug"] = hdrug
            return h_t

        # ---- software-pipelined emission ----
        a0 = phase_a(0)
        b0 = phase_b(0, a0, h_t)
        a1 = phase_a(1)
        h_t = phase_c(0, b0)
        b1 = phase_b(1, a1, h_t)
        a2 = phase_a(2)
        h_t = phase_c(1, b1)
        phase_b(2, a2, h_t)

        # ---- head: ypred = (a P1 P2) . (b P1) ----
        drug_rows = st8["drug_rows"]
        dT = misc_p.tile([P, ND3 * 2], FP, tag="dT", bufs=1)
        for d in range(ND3):
            pt = psum_p.tile([P, 2], FP, tag="ps", name="ptd")
            nc.tensor.transpose(pt[:], drug_rows[:, d * P:(d + 1) * P],
                                identf[0:2, 0:2])
            nc.vector.tensor_copy(dT[:, d * 2:(d + 1) * 2], pt[:])
        pw = psum_p.tile([P, 2], FP, tag="ps", name="pw")
        for d in range(ND3):
            nc.tensor.matmul(pw[:], p1_t[:, d * DEC:(d + 1) * DEC],
                             dT[:, d * 2:(d + 1) * 2],
                             start=(d == 0), stop=(d == ND3 - 1))
        w_sb = misc_p.tile([P, 2], FP, tag="w_sb", bufs=1)
        nc.vector.tensor_copy(w_sb[:], pw[:])
        ptt = psum_p.tile([P, 1], FP, tag="ps", name="ptt")
        nc.tensor.matmul(ptt[:], p2_t[:], w_sb[:, 0:1], start=True, stop=True)
        t_sb = misc_p.tile([P, 1], FP, tag="t_sb", bufs=1)
        nc.vector.tensor_copy(t_sb[:], ptt[:])
        py = psum_p.tile([1, 1], FP, tag="ps", name="py")
        nc.tensor.matmul(py[:], t_sb[:], w_sb[:, 1:2], start=True, stop=True)
        y_sb = misc_p.tile([1, 1], FP, tag="y_sb", bufs=1)
        nc.vector.tensor_copy(y_sb[:], py[:])
        nc.sync.dma_start(y_d.ap(), y_sb[:])

    nc.compile()
    return nc


# ---------------------------------------------------------------------------
# Host-side input prep
# ---------------------------------------------------------------------------

def make_in_maps(inputs: dict, n_cores: int):
    """Per-core input dicts. Core 2b = up path of batch b, 2b+1 = down."""
    bf = lambda a: np.ascontiguousarray(
        np.asarray(a, dtype=np.float32).astype(np.float16))

    def bake_mask(w):
        w = np.array(w, dtype=np.float32)
        w[-2:, :] = 1.0
        w[:, -2:] = 1.0
        return w
    maps = []
    for c in range(n_cores):
        b, down = divmod(c, 2)
        m = {
            "x": bf(inputs["x"][b]),
            "p1": np.ascontiguousarray(np.asarray(inputs["parameter1"], np.float32)),
            "p2": np.ascontiguousarray(np.asarray(inputs["parameter2"], np.float32)),
        }
        if not down:
            m["adj"] = bf(inputs["adj"][b])
            m["adj2"] = bf(np.asarray(inputs["adj"][b]).T)
            m["invT"] = bf(np.asarray(inputs["up_inv_deg"][b]).T)
            for l in range(3):
                wa = bake_mask(inputs[f"l{l}_up_adj_w"])
                m[f"w{l}a"] = bf(wa.T if l == 2 else wa)
                m[f"w{l}c"] = bf(inputs[f"l{l}_up_w"])
                m[f"w{l}b"] = bf(inputs[f"l{l}_bias"][:, :BH])
        else:
            m["adj"] = bf(np.asarray(inputs["adj"][b]).T)
            m["adj2"] = bf(inputs["adj"][b])
            m["invT"] = bf(np.asarray(inputs["down_inv_deg"][b]).T)
            for l in range(3):
                wa = bake_mask(np.asarray(inputs[f"l{l}_down_adj_w"]).T)
                m[f"w{l}a"] = bf(wa.T if l == 2 else wa)
                m[f"w{l}c"] = bf(inputs[f"l{l}_down_w"])
                m[f"w{l}b"] = bf(inputs[f"l{l}_bias"][:, BH:])
        maps.append(m)
    return maps


_nc_cache = {}


def _get_program(n_cores, N):
    key = (n_cores, N)
    if key not in _nc_cache:
        _nc_cache[key] = build_program(n_cores, N)
    return _nc_cache[key]


def kernel(**inputs) -> np.ndarray:
    n_cores = 8
    nc = _get_program(n_cores, N_FULL)
    in_maps = make_in_maps(inputs, n_cores)
    res = run_bass_kernel_spmd(nc, in_maps, core_ids=list(range(n_cores)))
    out = np.zeros((B, 1), dtype=np.float32)
    for b in range(B):
        out[b, 0] = res.results[2 * b]["ypred"][0, 0]
    return out
